# revision 11
# baseline (speedup 1.0000x reference)
"""Trainium2 Bass kernel for pairwise-similarity distillation loss.

Reference computes, per image i of the folded batch (B*L = 8 images,
each [C=32, HW=4096] after flattening space):

    That = T / (||T||_channels + eps);  Shat likewise
    loss = sum_i || That_i^T That_i - Shat_i^T Shat_i ||_F^2 / (HW^2 * B * L)

The HW x HW Gram matrices are never materialized.  With
V = [That; Shat] in R^{64 x HW} and J = diag(+1 x32, -1 x32):

    || G_T - G_S ||_F^2 = tr(J M J M),   M = V V^T  (64 x 64)

which is O(HW * 64^2) work instead of O(HW^2 * C) -- the kernel becomes
memory-bound (read 2 x 512KB per image).

Sharding: data-parallel over the 8 images, one per NeuronCore.  Each core
returns its scalar tr(JMJM) * 1/(HW^2*B*L); the host sums the 8 scalars.

The walrus build used here allows only ONE semaphore wait per compute
instruction (and a handful on the tail drain), so the kernel is structured
so no instruction ever needs to sync on two producers: constants ride in
the first data DMA (single queue wait), and every tile pool has one buffer
per group (no slot-reuse waits).
"""

import numpy as np
from contextlib import ExitStack

import concourse.bass as bass
import concourse.tile as tile
from concourse import bacc, mybir
from concourse.bass_utils import run_bass_kernel_spmd

F32 = mybir.dt.float32

N_CORES = 8
B, L, C, H, W = 2, 4, 32, 64, 64
HW = H * W            # 4096
C2 = 2 * C            # 64: T channels stacked on S channels
EPS = 1e-8
SCALE = 1.0 / (float(HW) * float(HW) * float(B) * float(L))

NGROUPS = 4                      # column groups for pipelining
GCOLS = HW // NGROUPS            # 1024 spatial positions per group
CHUNKS = GCOLS // 128            # 8 transpose chunks of 128 per group
CPRE = C2 + 1                    # const prefix columns: [identity | sgn]


def _emit(tc: tile.TileContext, out_ap, ts_in):
    nc = tc.nc
    with ExitStack() as ctx:
        in_pool = ctx.enter_context(tc.tile_pool(name="vraw", bufs=NGROUPS))
        pt_pool = ctx.enter_context(
            tc.tile_pool(name="pt", bufs=NGROUPS, space="PSUM")
        )
        acc_pool = ctx.enter_context(tc.tile_pool(name="acc", bufs=1, space="PSUM"))
        work = ctx.enter_context(tc.tile_pool(name="work", bufs=NGROUPS))

        mpsum = acc_pool.tile([C2, C2], F32, tag="m")
        id_tile = None
        sgn_tile = None

        for g in range(NGROUPS):
            # Raw [C2, GCOLS] slab: T channels on partitions 0:32, S on 32:64.
            # Group 0 additionally carries the [identity | sgn] const prefix
            # so PE's const dependency shares the data DMA's queue wait.
            if g == 0:
                vraw0 = in_pool.tile([C2, CPRE + GCOLS], F32, tag="vraw")
                nc.sync.dma_start(vraw0[:], ts_in[:, 0 : CPRE + GCOLS])
                id_tile = vraw0[:, 0:C2]
                sgn_tile = vraw0[:, C2 : C2 + 1]
                data = vraw0[:, CPRE : CPRE + GCOLS]
            else:
                vraw = in_pool.tile([C2, GCOLS], F32, tag="vraw")
                nc.sync.dma_start(
                    vraw[:], ts_in[:, CPRE + g * GCOLS : CPRE + (g + 1) * GCOLS]
                )
                data = vraw[:]

            # Transpose 128-column chunks into one PSUM bank:
            # pt[:, 64j:64j+64] = data[:, 128j:128j+128]^T
            pt = pt_pool.tile([128, CHUNKS * C2], F32, tag="pt")
            for j in range(CHUNKS):
                nc.tensor.transpose(
                    pt[:, bass.ts(j, C2)], data[:, bass.ts(j, 128)], id_tile
                )

            vt = work.tile([128, CHUNKS * C2], F32, tag="vt")
            nc.vector.tensor_copy(vt[:], pt[:])

            # Norms: group cols as [128, 2*CHUNKS groups, 32]; per-group sum of
            # squares -> n2[:, 2j] = T-half of chunk j, n2[:, 2j+1] = S-half.
            sq = work.tile([128, CHUNKS * C2], F32, tag="sq")
            nc.scalar.square(sq[:], vt[:])
            n2 = work.tile([128, 2 * CHUNKS], F32, tag="n2")
            nc.vector.reduce_sum(
                n2[:],
                sq[:].rearrange("p (g c) -> p g c", c=C),
                axis=mybir.AxisListType.X,
            )
            nrm = work.tile([128, 2 * CHUNKS], F32, tag="nrm")
            nc.scalar.sqrt(nrm[:], n2[:])
            nc.vector.tensor_scalar_add(nrm[:], nrm[:], EPS)
            r = work.tile([128, 2 * CHUNKS], F32, tag="r")
            nc.vector.reciprocal(r[:], nrm[:])

            # Normalize: vts[p, 32g + c] = vt[p, 32g + c] * r[p, g]
            vts = work.tile([128, CHUNKS * C2], F32, tag="vts")
            nc.vector.tensor_tensor(
                vts[:].rearrange("p (g c) -> p g c", c=C),
                vt[:].rearrange("p (g c) -> p g c", c=C),
                r[:].unsqueeze(2).broadcast_to((128, 2 * CHUNKS, C)),
                op=mybir.AluOpType.mult,
            )

            # Gram accumulation: M += vts_j^T @ vts_j over all chunks.
            for j in range(CHUNKS):
                nc.tensor.matmul(
                    mpsum[:],
                    vts[:, bass.ts(j, C2)],
                    vts[:, bass.ts(j, C2)],
                    start=(g == 0 and j == 0),
                    stop=(g == NGROUPS - 1 and j == CHUNKS - 1),
                )

        # loss = sum_ij s_i s_j M_ij^2  (s = +1 for T rows, -1 for S rows)
        msq = work.tile([C2, C2], F32, tag="msq")
        nc.scalar.square(msq[:], mpsum[:])
        asum = work.tile([C2, 1], F32, tag="asum")
        nc.vector.reduce_sum(asum[:], msq[:, 0:C], axis=mybir.AxisListType.X)
        bsum = work.tile([C2, 1], F32, tag="bsum")
        nc.vector.reduce_sum(bsum[:], msq[:, C:C2], axis=mybir.AxisListType.X)
        d = work.tile([C2, 1], F32, tag="d")
        nc.vector.tensor_tensor(d[:], asum[:], bsum[:], op=mybir.AluOpType.subtract)

        res_ps = acc_pool.tile([1, 1], F32, tag="res")
        nc.tensor.matmul(res_ps[:], d[:], sgn_tile, start=True, stop=True)
        res_sb = work.tile([1, 1], F32, tag="res_sb")
        nc.vector.tensor_copy(res_sb[:], res_ps[:])
        nc.sync.dma_start(out_ap, res_sb[:])


def build_nc(compile: bool = True) -> bass.Bass:
    nc = bacc.Bacc("TRN2", debug=False)
    ts_in = nc.dram_tensor("ts_in", [C2, CPRE + HW], F32, kind="ExternalInput").ap()
    out = nc.dram_tensor("out", [1, 1], F32, kind="ExternalOutput").ap()
    with tile.TileContext(nc) as tc:
        _emit(tc, out, ts_in)
    if compile:
        nc.compile()
    return nc


_NC_CACHE: bass.Bass | None = None


def _get_nc() -> bass.Bass:
    global _NC_CACHE
    if _NC_CACHE is None:
        _NC_CACHE = build_nc()
    return _NC_CACHE


def _const_prefix():
    # [identity | sgn] packed as [64, 65]; sgn carries the final loss scale.
    cst = np.zeros((C2, CPRE), dtype=np.float32)
    cst[:, 0:C2] = np.eye(C2, dtype=np.float32)
    cst[0:C, C2] = SCALE
    cst[C:C2, C2] = -SCALE
    return cst


def kernel(preds_S, preds_T) -> np.ndarray:
    S = np.asarray(preds_S, dtype=np.float32).reshape(B * L, C, HW)
    T = np.asarray(preds_T, dtype=np.float32).reshape(B * L, C, HW)
    TS = np.concatenate([T, S], axis=1)  # [8, 64, HW]
    cst = np.broadcast_to(_const_prefix(), (B * L, C2, CPRE))
    full = np.ascontiguousarray(np.concatenate([cst, TS], axis=2))  # [8,64,CPRE+HW]
    in_maps = [{"ts_in": full[i]} for i in range(N_CORES)]
    res = run_bass_kernel_spmd(_get_nc(), in_maps, list(range(N_CORES))).results
    total = np.float64(0.0)
    for i in range(N_CORES):
        total += np.float64(res[i]["out"].reshape(()))
    return np.float32(total)


# revision 12
# speedup vs baseline: 1.3653x; 1.3653x over previous
"""Trainium2 Bass kernel for pairwise-similarity distillation loss.

Reference computes, per image i of the folded batch (B*L = 8 images,
each [C=32, HW=4096] after flattening space):

    That = T / (||T||_channels + eps);  Shat likewise
    loss = sum_i || That_i^T That_i - Shat_i^T Shat_i ||_F^2 / (HW^2 * B * L)

The HW x HW Gram matrices are never materialized.  With
V = [That; Shat] in R^{64 x HW} and J = diag(+1 x32, -1 x32):

    || G_T - G_S ||_F^2 = tr(J M J M),   M = V V^T  (64 x 64)

which is O(HW * 64^2) work instead of O(HW^2 * C) -- the kernel becomes
memory-bound (read 2 x 512KB per image).

Sharding: data-parallel over the 8 images, one per NeuronCore.  Each core
returns its scalar tr(JMJM) * 1/(HW^2*B*L); the host sums the 8 scalars.

The walrus build used here allows only ONE semaphore wait per compute
instruction (and a handful on the tail drain), so the kernel is structured
so no instruction ever needs to sync on two producers: constants ride in
the first data DMA (single queue wait), and every tile pool has one buffer
per group (no slot-reuse waits).
"""

import numpy as np
from contextlib import ExitStack

import concourse.bass as bass
import concourse.tile as tile
from concourse import bacc, mybir
from concourse.bass_utils import run_bass_kernel_spmd

F32 = mybir.dt.float32

N_CORES = 8
B, L, C, H, W = 2, 4, 32, 64, 64
HW = H * W            # 4096
C2 = 2 * C            # 64: T channels stacked on S channels
EPS = 1e-8
SCALE = 1.0 / (float(HW) * float(HW) * float(B) * float(L))

NGROUPS = 4                      # column groups for pipelining
GCOLS = HW // NGROUPS            # 1024 spatial positions per group
CHUNKS = GCOLS // 128            # 8 transpose chunks of 128 per group
CPRE = C2 + 1                    # const prefix columns: [identity | sgn]


def _emit(tc: tile.TileContext, out_ap, ts_in):
    nc = tc.nc
    with ExitStack() as ctx:
        in_pool = ctx.enter_context(tc.tile_pool(name="vraw", bufs=NGROUPS))
        pt_pool = ctx.enter_context(
            tc.tile_pool(name="pt", bufs=NGROUPS, space="PSUM")
        )
        acc_pool = ctx.enter_context(tc.tile_pool(name="acc", bufs=1, space="PSUM"))
        work = ctx.enter_context(tc.tile_pool(name="work", bufs=NGROUPS))

        mpsum = acc_pool.tile([C2, C2], F32, tag="m")
        id_tile = None
        sgn_tile = None

        for g in range(NGROUPS):
            # Raw [C2, GCOLS] slab: T channels on partitions 0:32, S on 32:64.
            # Group 0 additionally carries the [identity | sgn] const prefix
            # so PE's const dependency shares the data DMA's queue wait.
            if g == 0:
                vraw0 = in_pool.tile([C2, CPRE + GCOLS], F32, tag="vraw")
                nc.sync.dma_start(vraw0[:], ts_in[:, 0 : CPRE + GCOLS])
                id_tile = vraw0[:, 0:C2]
                sgn_tile = vraw0[:, C2 : C2 + 1]
                data = vraw0[:, CPRE : CPRE + GCOLS]
            else:
                vraw = in_pool.tile([C2, GCOLS], F32, tag="vraw")
                nc.sync.dma_start(
                    vraw[:], ts_in[:, CPRE + g * GCOLS : CPRE + (g + 1) * GCOLS]
                )
                data = vraw[:]

            # Transpose 128-column chunks into one PSUM bank:
            # pt[:, 64j:64j+64] = data[:, 128j:128j+128]^T
            pt = pt_pool.tile([128, CHUNKS * C2], F32, tag="pt")
            for j in range(CHUNKS):
                nc.tensor.transpose(
                    pt[:, bass.ts(j, C2)], data[:, bass.ts(j, 128)], id_tile
                )

            vt = work.tile([128, CHUNKS * C2], F32, tag="vt")
            nc.vector.tensor_copy(vt[:], pt[:])

            # Norms: group cols as [128, 2*CHUNKS groups, 32]; per-group sum of
            # squares -> n2[:, 2j] = T-half of chunk j, n2[:, 2j+1] = S-half.
            sq = work.tile([128, CHUNKS * C2], F32, tag="sq")
            nc.scalar.square(sq[:], vt[:])
            n2 = work.tile([128, 2 * CHUNKS], F32, tag="n2")
            nc.vector.reduce_sum(
                n2[:],
                sq[:].rearrange("p (g c) -> p g c", c=C),
                axis=mybir.AxisListType.X,
            )
            nrm = work.tile([128, 2 * CHUNKS], F32, tag="nrm")
            nc.scalar.sqrt(nrm[:], n2[:])
            nc.vector.tensor_scalar_add(nrm[:], nrm[:], EPS)
            r = work.tile([128, 2 * CHUNKS], F32, tag="r")
            nc.vector.reciprocal(r[:], nrm[:])

            # Normalize: vts[p, 32g + c] = vt[p, 32g + c] * r[p, g]
            vts = work.tile([128, CHUNKS * C2], F32, tag="vts")
            nc.vector.tensor_tensor(
                vts[:].rearrange("p (g c) -> p g c", c=C),
                vt[:].rearrange("p (g c) -> p g c", c=C),
                r[:].unsqueeze(2).broadcast_to((128, 2 * CHUNKS, C)),
                op=mybir.AluOpType.mult,
            )

            # Gram accumulation: M += vts_j^T @ vts_j over all chunks.
            for j in range(CHUNKS):
                nc.tensor.matmul(
                    mpsum[:],
                    vts[:, bass.ts(j, C2)],
                    vts[:, bass.ts(j, C2)],
                    start=(g == 0 and j == 0),
                    stop=(g == NGROUPS - 1 and j == CHUNKS - 1),
                )

        # loss = sum_ij s_i s_j M_ij^2  (s = +1 for T rows, -1 for S rows)
        msq = work.tile([C2, C2], F32, tag="msq")
        nc.scalar.square(msq[:], mpsum[:])
        asum = work.tile([C2, 1], F32, tag="asum")
        nc.vector.reduce_sum(asum[:], msq[:, 0:C], axis=mybir.AxisListType.X)
        bsum = work.tile([C2, 1], F32, tag="bsum")
        nc.vector.reduce_sum(bsum[:], msq[:, C:C2], axis=mybir.AxisListType.X)
        d = work.tile([C2, 1], F32, tag="d")
        nc.vector.tensor_tensor(d[:], asum[:], bsum[:], op=mybir.AluOpType.subtract)

        res_ps = acc_pool.tile([1, 1], F32, tag="res")
        nc.tensor.matmul(res_ps[:], d[:], sgn_tile, start=True, stop=True)
        res_sb = work.tile([1, 1], F32, tag="res_sb")
        nc.vector.tensor_copy(res_sb[:], res_ps[:])
        nc.sync.dma_start(out_ap, res_sb[:])


STAIR = [4, 4, 8, 8, 4, 4]


def build_nc(compile: bool = True) -> bass.Bass:
    from kernel_emit_v3 import emit_v3

    nc = bacc.Bacc("TRN2", debug=False)
    ts_in = nc.dram_tensor("ts_in", [C2, CPRE + HW], F32, kind="ExternalInput").ap()
    out = nc.dram_tensor("out", [1, 1], F32, kind="ExternalOutput").ap()
    with tile.TileContext(nc) as tc:
        emit_v3(tc, out, ts_in, stair=STAIR)
    if compile:
        nc.compile()
    return nc


_NC_CACHE: bass.Bass | None = None


def _get_nc() -> bass.Bass:
    global _NC_CACHE
    if _NC_CACHE is None:
        _NC_CACHE = build_nc()
    return _NC_CACHE


def _const_prefix():
    # [identity | sgn] packed as [64, 65]; sgn carries the final loss scale.
    cst = np.zeros((C2, CPRE), dtype=np.float32)
    cst[:, 0:C2] = np.eye(C2, dtype=np.float32)
    cst[0:C, C2] = SCALE
    cst[C:C2, C2] = -SCALE
    return cst


def kernel(preds_S, preds_T) -> np.ndarray:
    S = np.asarray(preds_S, dtype=np.float32).reshape(B * L, C, HW)
    T = np.asarray(preds_T, dtype=np.float32).reshape(B * L, C, HW)
    TS = np.concatenate([T, S], axis=1)  # [8, 64, HW]
    cst = np.broadcast_to(_const_prefix(), (B * L, C2, CPRE))
    full = np.ascontiguousarray(np.concatenate([cst, TS], axis=2))  # [8,64,CPRE+HW]
    in_maps = [{"ts_in": full[i]} for i in range(N_CORES)]
    res = run_bass_kernel_spmd(_get_nc(), in_maps, list(range(N_CORES))).results
    total = np.float64(0.0)
    for i in range(N_CORES):
        total += np.float64(res[i]["out"].reshape(()))
    return np.float32(total)


# revision 13
# speedup vs baseline: 1.3710x; 1.0042x over previous
"""Trainium2 Bass kernel for pairwise-similarity distillation loss.

Reference computes, per image i of the folded batch (B*L = 8 images,
each [C=32, HW=4096] after flattening space):

    That = T / (||T||_channels + eps);  Shat likewise
    loss = sum_i || That_i^T That_i - Shat_i^T Shat_i ||_F^2 / (HW^2 * B * L)

The HW x HW Gram matrices are never materialized.  With
V = [That; Shat] in R^{64 x HW} and J = diag(+1 x32, -1 x32):

    || G_T - G_S ||_F^2 = tr(J M J M),   M = V V^T  (64 x 64)

which is O(HW * 64^2) work instead of O(HW^2 * C) -- the kernel becomes
memory-bound (read 2 x 512KB per image).

Sharding: data-parallel over the 8 images, one per NeuronCore.  Each core
returns its scalar tr(JMJM) * 1/(HW^2*B*L); the host sums the 8 scalars.

Per-core dataflow (Tile framework schedules all sync):
  - staircase column groups pipeline DMA -> PE transpose -> norms -> Gram
  - PE transposes run at high priority so every group's PSUM bank is ready
    early and the ACT/DVE norm chains overlap across groups
  - channel norms are computed in the transposed domain (ACT square from
    PSUM, DVE grouped reduce, ACT sqrt, DVE reciprocal) and applied with a
    single broadcast multiply reading the transpose result straight from
    PSUM (only one PSUM operand per DVE op is allowed)
  - the identity (for PE transpose) and the signed/scaled J vector ride as
    a 65-column prefix of the group-0 DMA, so no instruction needs more
    than one semaphore wait (this walrus build allows only one per compute
    instruction; bacc.compile() legalizes the rest)
"""

import numpy as np
from contextlib import ExitStack

import concourse.bass as bass
import concourse.tile as tile
from concourse import bacc, mybir
from concourse.bass_utils import run_bass_kernel_spmd

F32 = mybir.dt.float32

N_CORES = 8
B, L, C, H, W = 2, 4, 32, 64, 64
HW = H * W            # 4096
C2 = 2 * C            # 64: T channels stacked on S channels
SCALE = 1.0 / (float(HW) * float(HW) * float(B) * float(L))
CPRE = C2 + 1         # const prefix columns: [identity | sgn]

# chunks (128 spatial cols each) per DMA/compute group; sum must be 32
STAIR = [4, 6, 6, 6, 6, 4]


def _emit(tc: tile.TileContext, out_ap, ts_in, stair):
    nc = tc.nc
    assert sum(stair) == 32 and all(n <= 8 for n in stair)
    ngr = len(stair)
    with ExitStack() as ctx:
        in_pool = ctx.enter_context(tc.tile_pool(name="vraw", bufs=ngr))
        pt_pool = ctx.enter_context(
            tc.tile_pool(name="pt", bufs=min(ngr, 6), space="PSUM")
        )
        acc_pool = ctx.enter_context(tc.tile_pool(name="acc", bufs=1, space="PSUM"))
        work = ctx.enter_context(tc.tile_pool(name="work", bufs=ngr))

        # Prefetch the ACT function table (Square/Sqrt) while DMAs run, so
        # the first real sqrt doesn't stall ~1.3us on LoadActFuncSet.
        warm_in = work.tile([1, 2], F32, tag="warm_in")
        nc.gpsimd.memset(warm_in[:], 1.0)
        warm_out = work.tile([1, 2], F32, tag="warm_out")
        nc.scalar.square(warm_out[:, 0:1], warm_in[:, 0:1])
        nc.scalar.sqrt(warm_out[:, 1:2], warm_in[:, 1:2])

        mpsum = acc_pool.tile([C2, C2], F32, tag="m")
        id_tile = None
        sgn_tile = None

        first = True
        off = 0
        for g, n in enumerate(stair):
            cols = 128 * n
            # Raw [C2, cols] slab: T channels on partitions 0:32, S on 32:64.
            # Group 0 additionally carries the [identity | sgn] const prefix
            # so PE's const dependency shares the data DMA's queue wait.
            if g == 0:
                vraw0 = in_pool.tile([C2, CPRE + cols], F32, tag="vraw")
                nc.sync.dma_start(vraw0[:], ts_in[:, 0 : CPRE + cols])
                id_tile = vraw0[:, 0:C2]
                sgn_tile = vraw0[:, C2 : C2 + 1]
                data = vraw0[:, CPRE : CPRE + cols]
            else:
                vraw = in_pool.tile([C2, cols], F32, tag="vraw")
                nc.sync.dma_start(
                    vraw[:], ts_in[:, CPRE + off : CPRE + off + cols]
                )
                data = vraw[:]
            off += cols

            # Transposes run at max priority: PE prefers them over queued
            # Gram matmuls, so pt banks (and thus ACT squares) are ready
            # early and the per-group norm chains overlap across groups.
            pt = pt_pool.tile([128, C2 * n], F32, tag="pt")
            with tc.high_priority():
                for j in range(n):
                    nc.tensor.transpose(
                        pt[:, bass.ts(j, C2)], data[:, bass.ts(j, 128)], id_tile
                    )

            # Norms: view cols as [128, 2n groups, 32]; n2[:, 2j] = T-half of
            # chunk j, n2[:, 2j+1] = S-half.  (eps=1e-8 of the reference is
            # below fp32 ULP at these magnitudes and is dropped.)
            sq = work.tile([128, C2 * n], F32, tag="sq")
            nc.scalar.square(sq[:], pt[:])
            n2 = work.tile([128, 2 * n], F32, tag="n2")
            nc.vector.reduce_sum(
                n2[:],
                sq[:].rearrange("p (g c) -> p g c", c=C),
                axis=mybir.AxisListType.X,
            )
            nrm = work.tile([128, 2 * n], F32, tag="nrm")
            nc.scalar.sqrt(nrm[:], n2[:])
            r = work.tile([128, 2 * n], F32, tag="r")
            nc.vector.reciprocal(r[:], nrm[:])

            # Normalize straight from PSUM: vts[p, 32g+c] = pt[p, 32g+c]*r[p, g]
            vts = work.tile([128, C2 * n], F32, tag="vts")
            nc.vector.tensor_tensor(
                vts[:].rearrange("p (g c) -> p g c", c=C),
                pt[:].rearrange("p (g c) -> p g c", c=C),
                r[:].unsqueeze(2).broadcast_to((128, 2 * n, C)),
                op=mybir.AluOpType.mult,
            )

            # Gram accumulation: M += vts_j^T @ vts_j over all chunks.
            for j in range(n):
                nc.tensor.matmul(
                    mpsum[:],
                    vts[:, bass.ts(j, C2)],
                    vts[:, bass.ts(j, C2)],
                    start=first,
                    stop=(g == ngr - 1 and j == n - 1),
                )
                first = False

        # loss = sum_ij s_i s_j M_ij^2  (s = +1 for T rows, -1 for S rows):
        # row-group sums of M^2, signed subtract, then a [64]x[64,1] matmul
        # against the scaled sign vector collapses the partition dim.
        msq = work.tile([C2, C2], F32, tag="msq")
        nc.scalar.square(msq[:], mpsum[:])
        ab = work.tile([C2, 2], F32, tag="ab")
        nc.vector.reduce_sum(
            ab[:],
            msq[:].rearrange("p (g c) -> p g c", c=C),
            axis=mybir.AxisListType.X,
        )
        d = work.tile([C2, 1], F32, tag="d")
        nc.vector.tensor_tensor(
            d[:], ab[:, 0:1], ab[:, 1:2], op=mybir.AluOpType.subtract
        )

        res_ps = acc_pool.tile([1, 1], F32, tag="res")
        nc.tensor.matmul(res_ps[:], d[:], sgn_tile, start=True, stop=True)
        res_sb = work.tile([1, 1], F32, tag="res_sb")
        nc.vector.tensor_copy(res_sb[:], res_ps[:])
        nc.sync.dma_start(out_ap, res_sb[:])


def build_nc(compile: bool = True) -> bass.Bass:
    nc = bacc.Bacc("TRN2", debug=False)
    ts_in = nc.dram_tensor("ts_in", [C2, CPRE + HW], F32, kind="ExternalInput").ap()
    out = nc.dram_tensor("out", [1, 1], F32, kind="ExternalOutput").ap()
    with tile.TileContext(nc) as tc:
        _emit(tc, out, ts_in, STAIR)
    if compile:
        nc.compile()
    return nc


_NC_CACHE: bass.Bass | None = None


def _get_nc() -> bass.Bass:
    global _NC_CACHE
    if _NC_CACHE is None:
        _NC_CACHE = build_nc()
    return _NC_CACHE


def _const_prefix():
    # [identity | sgn] packed as [64, 65]; sgn carries the final loss scale.
    cst = np.zeros((C2, CPRE), dtype=np.float32)
    cst[:, 0:C2] = np.eye(C2, dtype=np.float32)
    cst[0:C, C2] = SCALE
    cst[C:C2, C2] = -SCALE
    return cst


def kernel(preds_S, preds_T) -> np.ndarray:
    S = np.asarray(preds_S, dtype=np.float32).reshape(B * L, C, HW)
    T = np.asarray(preds_T, dtype=np.float32).reshape(B * L, C, HW)
    TS = np.concatenate([T, S], axis=1)  # [8, 64, HW]
    cst = np.broadcast_to(_const_prefix(), (B * L, C2, CPRE))
    full = np.ascontiguousarray(np.concatenate([cst, TS], axis=2))
    in_maps = [{"ts_in": full[i]} for i in range(N_CORES)]
    res = run_bass_kernel_spmd(_get_nc(), in_maps, list(range(N_CORES))).results
    total = np.float64(0.0)
    for i in range(N_CORES):
        total += np.float64(res[i]["out"].reshape(()))
    return np.float32(total)
